# revision 13
# baseline (speedup 1.0000x reference)
"""Trainium2 Bass kernel for a 4-layer GPT-style transformer (B=2, T=2048,
D=512, H=8, V=32000) returning (loss, logits), distributed over 8 NeuronCores.

Sharding strategy (self-contained, hardcoded):
  - Trunk: sequence-parallel. Core c owns 512 tokens: batch c//4, token block
    [512*(c%4), 512*(c%4+1)). Activations are kept feature-major in SBUF
    (xT[d, t]) so every matmul contraction sits on the partition axis with no
    transposes.
  - Attention: head-parallel. Core c computes head c for BOTH batches over all
    2048 tokens. q/k/v are exchanged with one 8-core AllToAll each (core p
    sends head q of its tokens to core q); attention outputs return with a
    fourth AllToAll. Perfectly balanced and the program is identical on every
    core (SPMD-safe).
  - LM head: vocab-parallel. After a final AllGather of the trunk output, core
    c computes logits[:, 4000*c : 4000*(c+1)] plus a per-shard sum(exp(logit))
    for the loss. Host combines shards: logsumexp + target gather + mean.

Precision: all matmul operands are bf16 (PE runs bf16 at 1 cycle/row vs 4 for
fp32, and fp32 PSUM accumulation is broken on this toolchain); the residual
stream, PSUM accumulation, softmax statistics and logits stay fp32.
"""

import numpy as np
import ml_dtypes

import concourse.bass as bass
import concourse.bacc as bacc
import concourse.tile as tile
from concourse import mybir
from concourse.bass_utils import run_bass_kernel_spmd
from concourse.masks import make_identity

# model dims (hardcoded per the problem spec)
B, T, V, D, H, HS, L, FF = 2, 2048, 32000, 512, 8, 64, 4, 2048
EPS = 1e-5
NCORES = 8
TOK = 512          # tokens per core
VS = V // NCORES   # vocab shard = 4000
VC = 500           # vocab chunk (8 chunks per shard)
SCALE = 1.0 / float(np.sqrt(np.float32(D)))

f32 = mybir.dt.float32
bf16 = mybir.dt.bfloat16
i16 = mybir.dt.int16
AF = mybir.ActivationFunctionType
ALU = mybir.AluOpType

import os as _os
NUM_LAYERS = int(_os.environ.get("KRN_LAYERS", L))  # reduce for debugging

_CACHE = {}


def _build():
    """Build + compile the single SPMD program (same on all 8 cores)."""
    nc = bacc.Bacc("TRN2", target_bir_lowering=False, debug=False,
                   num_devices=NCORES)

    # ---------------- DRAM I/O ----------------
    tok_emb = nc.dram_tensor("tok_emb", [V, D], f32, kind="ExternalInput")
    idxw = nc.dram_tensor("idxw", [128, TOK // 16], i16, kind="ExternalInput")
    pos_tm = nc.dram_tensor("pos_tm", [128, 4, D], f32, kind="ExternalInput")
    wq = nc.dram_tensor("wq", [L, 4, 128, H * HS], bf16, kind="ExternalInput")
    wk = nc.dram_tensor("wk", [L, 4, 128, H * HS], bf16, kind="ExternalInput")
    wv = nc.dram_tensor("wv", [L, 4, 128, H * HS], bf16, kind="ExternalInput")
    wo = nc.dram_tensor("wo", [L, 4, 128, D], bf16, kind="ExternalInput")
    w1 = nc.dram_tensor("w1", [L, 4, 128, FF], bf16, kind="ExternalInput")
    w2 = nc.dram_tensor("w2", [L, 16, 128, D], bf16, kind="ExternalInput")
    ln1g = nc.dram_tensor("ln1g", [L, 4, 128], f32, kind="ExternalInput")
    ln1b = nc.dram_tensor("ln1b", [L, 4, 128], f32, kind="ExternalInput")
    ln2g = nc.dram_tensor("ln2g", [L, 4, 128], f32, kind="ExternalInput")
    ln2b = nc.dram_tensor("ln2b", [L, 4, 128], f32, kind="ExternalInput")
    lnfg = nc.dram_tensor("lnfg", [4, 128], f32, kind="ExternalInput")
    lnfb = nc.dram_tensor("lnfb", [4, 128], f32, kind="ExternalInput")
    bo_in = nc.dram_tensor("bo_in", [L, 4, 128], f32, kind="ExternalInput")
    b1_in = nc.dram_tensor("b1_in", [L, 16, 128], f32, kind="ExternalInput")
    b2_in = nc.dram_tensor("b2_in", [L, 4, 128], f32, kind="ExternalInput")
    masks_in = nc.dram_tensor("masks_in", [4, 128, 512], bf16, kind="ExternalInput")
    wout = nc.dram_tensor("wout", [4, 128, VS], bf16, kind="ExternalInput")
    bout_in = nc.dram_tensor("bout_in", [VS], bf16, kind="ExternalInput")

    logits_out = nc.dram_tensor("logits_out", [B * T, VS], f32, kind="ExternalOutput")
    sumexp_out = nc.dram_tensor("sumexp_out", [32, 128], f32, kind="ExternalOutput")

    GRP = [list(range(NCORES))]

    with tile.TileContext(nc) as tc:
        import contextlib
        with contextlib.ExitStack() as ctx:
            const = ctx.enter_context(tc.tile_pool(name="const", bufs=1))
            dram = ctx.enter_context(tc.tile_pool(name="dram", bufs=2, space="DRAM"))
            ps = ctx.enter_context(tc.tile_pool(name="ps", bufs=4, space="PSUM"))
            ps_pv = ctx.enter_context(tc.tile_pool(name="ps_pv", bufs=2, space="PSUM"))
            ps_aux = ctx.enter_context(tc.tile_pool(name="ps_aux", bufs=2, space="PSUM"))

            # ---------------- constants ----------------
            ident = const.tile([128, 128], f32)
            make_identity(nc, ident[:])
            ones_row = const.tile([1, 512], f32)
            nc.vector.memset(ones_row[:], 1.0)
            ones_row_bf = const.tile([1, 128], bf16)
            nc.vector.memset(ones_row_bf[:], 1.0)
            invd = const.tile([128, 1], bf16)
            nc.vector.memset(invd[:], 1.0 / D)
            eps_sb = const.tile([1, 1], f32)
            nc.vector.memset(eps_sb[:], EPS)
            masks_sb = const.tile([128, 4, 512], bf16)
            nc.sync.dma_start(masks_sb[:], masks_in.ap().rearrange("m p f -> p m f"))
            idx_sb = const.tile([128, TOK // 16], i16)
            nc.sync.dma_start(idx_sb[:], idxw.ap())

            # residual stream, feature-major: x[d, t], d = 128*dc + p
            xT = const.tile([128, 4, 512], f32)

            # ---------------- embedding ----------------
            with tc.tile_pool(name="embp", bufs=1) as embp:
                emb_tm = embp.tile([128, 4, 512], f32, tag="emb")
                nc.gpsimd.dma_gather(emb_tm[:], tok_emb.ap(), idx_sb[:], TOK, TOK, D)
                pos_sb = embp.tile([128, 4, 512], f32, tag="pos")
                nc.sync.dma_start(pos_sb[:], pos_tm.ap())
                nc.vector.tensor_add(emb_tm[:], emb_tm[:], pos_sb[:])
                for ti in range(4):
                    for dc in range(4):
                        tp = ps.tile([128, 512], f32, tag="big", name=f"tp_{ti}_{dc}")
                        nc.tensor.transpose(tp[:, :128], emb_tm[:, ti, dc * 128:(dc + 1) * 128], ident[:])
                        nc.vector.tensor_copy(xT[:, dc, ti * 128:(ti + 1) * 128], tp[:, :128])

            trunk_ctx = contextlib.ExitStack()
            sb = trunk_ctx.enter_context(tc.tile_pool(name="sb", bufs=2))
            sb2 = trunk_ctx.enter_context(tc.tile_pool(name="sb2", bufs=2))
            wp = trunk_ctx.enter_context(tc.tile_pool(name="wp", bufs=1))

            # ---------------- helpers ----------------
            def layer_norm(g_ap, b_ap, out_bf_tile):
                """LN over feature axis of xT -> out_bf_tile [128,4,512] bf16.
                g_ap/b_ap: DRAM APs [4,128] for gain/bias."""
                gb = sb.tile([128, 4, 2], f32, tag="lngb")
                nc.sync.dma_start(gb[:, :, 0], g_ap.rearrange("c p -> p c"))
                nc.sync.dma_start(gb[:, :, 1], b_ap.rearrange("c p -> p c"))
                st_m = ps_aux.tile([128, 512], f32, tag="aux", name="st_m")
                st_q = ps_aux.tile([128, 512], f32, tag="aux", name="st_q")
                for dc in range(4):
                    xbf = sb.tile([128, 512], bf16, tag="xbf", name=f"xbf{dc}")
                    sqbf = sb.tile([128, 512], bf16, tag="sqbf", name=f"sqbf{dc}")
                    nc.scalar.copy(xbf[:], xT[:, dc, :])
                    nc.scalar.activation(sqbf[:], xT[:, dc, :], func=AF.Square)
                    nc.tensor.matmul(st_m[:1, :], invd[:], xbf[:],
                                     start=(dc == 0), stop=(dc == 3))
                    nc.tensor.matmul(st_q[:1, :], invd[:], sqbf[:],
                                     start=(dc == 0), stop=(dc == 3))
                mean_sb = sb.tile([1, 512], f32, tag="lnsmall", bufs=5)
                nc.vector.tensor_copy(mean_sb[:], st_m[:1, :])
                m2 = sb.tile([1, 512], f32, tag="lnsmall", bufs=5)
                nc.vector.tensor_mul(m2[:], mean_sb[:], mean_sb[:])
                var = sb.tile([1, 512], f32, tag="lnsmall", bufs=5)
                nc.vector.tensor_sub(var[:], st_q[:1, :], m2[:])
                lnv = sb.tile([1, 512], f32, tag="lnsmall", bufs=5)
                nc.scalar.activation(lnv[:], var[:], func=AF.Ln, bias=eps_sb[:])
                rstd = sb.tile([1, 512], f32, tag="lnsmall", bufs=5)
                nc.scalar.activation(rstd[:], lnv[:], func=AF.Exp, scale=-0.5)
                mu_bc = ps_aux.tile([128, 512], f32, tag="aux", name="mu_bc")
                rs_bc = ps_aux.tile([128, 512], f32, tag="aux", name="rs_bc")
                nc.tensor.matmul(mu_bc[:], ones_row[:, :128], mean_sb[:],
                                 start=True, stop=True)
                nc.tensor.matmul(rs_bc[:], ones_row[:, :128], rstd[:],
                                 start=True, stop=True)
                for dc in range(4):
                    t1 = sb.tile([128, 512], f32, tag="lnt1", name=f"lnt1_{dc}")
                    nc.vector.tensor_sub(t1[:], xT[:, dc, :], mu_bc[:])
                    nc.vector.tensor_mul(t1[:], t1[:], rs_bc[:])
                    nc.vector.tensor_scalar(out_bf_tile[:, dc, :], t1[:],
                                            gb[:, dc, 0:1], gb[:, dc, 1:2],
                                            ALU.mult, ALU.add)

            # ---------------- transformer layers ----------------
            for l in range(NUM_LAYERS):
                # ---- LN1 ----
                hT = sb2.tile([128, 4, 512], bf16, tag="hT", name=f"hT_{l}")
                layer_norm(ln1g.ap()[l], ln1b.ap()[l], hT)

                # ---- qkv + stage for AllToAll ----
                wq_sb = wp.tile([128, 4, 512], bf16, tag="wq", name=f"wq_{l}")
                wk_sb = wp.tile([128, 4, 512], bf16, tag="wk", name=f"wk_{l}")
                wv_sb = wp.tile([128, 4, 512], bf16, tag="wv", name=f"wv_{l}")
                nc.sync.dma_start(wq_sb[:], wq.ap()[l].rearrange("c p m -> p c m"))
                nc.sync.dma_start(wk_sb[:], wk.ap()[l].rearrange("c p m -> p c m"))
                nc.sync.dma_start(wv_sb[:], wv.ap()[l].rearrange("c p m -> p c m"))

                a2a_q_in = dram.tile([8, 64, 512], bf16, tag="a2a_qk", name=f"qin_{l}")
                a2a_k_in = dram.tile([8, 64, 512], bf16, tag="a2a_qk", name=f"kin_{l}")
                a2a_v_in = dram.tile([8, 512, 64], bf16, tag="a2a_v", name=f"vin_{l}")

                for p4 in range(4):
                    for (w_sb, dst) in ((wq_sb, a2a_q_in), (wk_sb, a2a_k_in)):
                        qk_ps = ps.tile([128, 512], f32, tag="big", name=f"qk_{l}_{p4}")
                        for kc in range(4):
                            nc.tensor.matmul(qk_ps[:], w_sb[:, kc, p4 * 128:(p4 + 1) * 128],
                                             hT[:, kc, :], start=(kc == 0), stop=(kc == 3))
                        qk_sb = sb.tile([128, 512], bf16, tag="qkst", name=f"qks_{l}_{p4}")
                        nc.scalar.copy(qk_sb[:], qk_ps[:])
                        nc.sync.dma_start(
                            dst[2 * p4:2 * p4 + 2].rearrange("d e t -> (d e) t"), qk_sb[:])
                    # v: token-major [tok, 2 heads * 64]
                    for tc4 in range(4):
                        v_ps = ps.tile([128, 512], f32, tag="big", name=f"v_{l}_{p4}_{tc4}")
                        for kc in range(4):
                            nc.tensor.matmul(v_ps[:, :128], hT[:, kc, tc4 * 128:(tc4 + 1) * 128],
                                             wv_sb[:, kc, p4 * 128:(p4 + 1) * 128],
                                             start=(kc == 0), stop=(kc == 3))
                        v_sb = sb.tile([128, 128], bf16, tag="vst", name=f"vs_{l}_{p4}_{tc4}")
                        nc.scalar.copy(v_sb[:], v_ps[:, :128])
                        nc.sync.dma_start(
                            a2a_v_in[2 * p4:2 * p4 + 2, tc4 * 128:(tc4 + 1) * 128, :]
                            .rearrange("d t e -> t d e"),
                            v_sb[:].rearrange("t (d e) -> t d e", d=2))

                a2a_q_out = dram.tile([8, 64, 512], bf16, tag="a2a_qko", name=f"qout_{l}")
                a2a_k_out = dram.tile([8, 64, 512], bf16, tag="a2a_qko", name=f"kout_{l}")
                a2a_v_out = dram.tile([8, 512, 64], bf16, tag="a2a_vo", name=f"vout_{l}")
                nc.gpsimd.collective_compute("AllToAll", ALU.bypass, replica_groups=GRP,
                                             ins=[a2a_q_in.opt()], outs=[a2a_q_out.opt()])
                nc.gpsimd.collective_compute("AllToAll", ALU.bypass, replica_groups=GRP,
                                             ins=[a2a_k_in.opt()], outs=[a2a_k_out.opt()])
                nc.gpsimd.collective_compute("AllToAll", ALU.bypass, replica_groups=GRP,
                                             ins=[a2a_v_in.opt()], outs=[a2a_v_out.opt()])

                # ---- attention: my head, both batches ----
                a2a_o_in = dram.tile([8, 64, 512], bf16, tag="a2a_qk", name=f"oin_{l}")
                for b2 in range(2):
                    kT_sb = sb2.tile([64, 4, 512], bf16, tag="kT", name=f"kT_{l}_{b2}")
                    qT_sb = sb2.tile([64, 4, 512], bf16, tag="qT", name=f"qT_{l}_{b2}")
                    nc.sync.dma_start(kT_sb[:], a2a_k_out[4 * b2:4 * b2 + 4]
                                      .rearrange("s p f -> p s f"))
                    nc.sync.dma_start(qT_sb[:], a2a_q_out[4 * b2:4 * b2 + 4]
                                      .rearrange("s p f -> p s f"))
                    v_g = sb2.tile([128, 16, 65], bf16, tag="vg", name=f"vg_{l}_{b2}")
                    nc.vector.memset(v_g[:, :, 64:65], 1.0)
                    nc.sync.dma_start(
                        v_g[:, :, :64],
                        a2a_v_out[4 * b2:4 * b2 + 4].rearrange("j (s p) e -> p (j s) e", p=128))
                    for qc in range(4):
                        pv_ps = ps_pv.tile([128, 512], f32, tag="pv", name=f"pv_{l}_{b2}_{qc}")
                        nsj = 4 * qc + 4
                        for sj in range(nsj):
                            s_ps = ps.tile([128, 512], f32, tag="big",
                                           name=f"s_{l}_{b2}_{qc}_{sj}")
                            nc.tensor.matmul(s_ps[:], kT_sb[:, sj // 4, (sj % 4) * 128:(sj % 4 + 1) * 128],
                                             qT_sb[:, qc, :], start=True, stop=True)
                            p_sb = sb.tile([128, 512], bf16, tag="psm",
                                           name=f"p_{l}_{b2}_{qc}_{sj}")
                            nc.scalar.activation(p_sb[:], s_ps[:], func=AF.Exp, scale=SCALE)
                            m = sj - 4 * qc
                            if m >= 0:
                                nc.vector.tensor_mul(p_sb[:], p_sb[:], masks_sb[:, m, :])
                            nc.tensor.matmul(pv_ps[:65, :], v_g[:, sj, :], p_sb[:],
                                             start=(sj == 0), stop=(sj == nsj - 1))
                        rec = sb.tile([1, 512], f32, tag="rec", name=f"rec_{l}_{b2}_{qc}")
                        nc.vector.reciprocal(rec[:], pv_ps[64:65, :])
                        ob_ps = ps.tile([128, 512], f32, tag="big", name=f"ob_{l}_{b2}_{qc}")
                        nc.tensor.matmul(ob_ps[:64, :], ones_row[:, :64], rec[:],
                                         start=True, stop=True)
                        ob_sb = sb.tile([64, 512], f32, tag="obst", name=f"obs_{l}_{b2}_{qc}")
                        nc.scalar.copy(ob_sb[:], ob_ps[:64, :])
                        o_sb = sb.tile([64, 512], bf16, tag="ost", name=f"o_{l}_{b2}_{qc}")
                        nc.vector.tensor_mul(o_sb[:], pv_ps[:64, :], ob_sb[:])
                        nc.sync.dma_start(a2a_o_in[4 * b2 + qc], o_sb[:])

                a2a_o_out = dram.tile([8, 64, 512], bf16, tag="a2a_qko", name=f"oout_{l}")
                nc.gpsimd.collective_compute("AllToAll", ALU.bypass, replica_groups=GRP,
                                             ins=[a2a_o_in.opt()], outs=[a2a_o_out.opt()])

                # ---- output projection + residual ----
                attnT = sb2.tile([128, 4, 512], bf16, tag="attnT", name=f"attnT_{l}")
                nc.sync.dma_start(attnT[:], a2a_o_out.rearrange("(j h2) e t -> (h2 e) j t", h2=2))
                wo_sb = wp.tile([128, 4, 512], bf16, tag="wo", name=f"wo_{l}")
                nc.sync.dma_start(wo_sb[:], wo.ap()[l].rearrange("c p m -> p c m"))
                bo_sb = sb.tile([128, 4], f32, tag="bo", name=f"bo_{l}")
                nc.sync.dma_start(bo_sb[:], bo_in.ap()[l].rearrange("c p -> p c"))
                for dc in range(4):
                    y_ps = ps.tile([128, 512], f32, tag="big", name=f"y_{l}_{dc}")
                    for kc in range(4):
                        nc.tensor.matmul(y_ps[:], wo_sb[:, kc, dc * 128:(dc + 1) * 128],
                                         attnT[:, kc, :], start=(kc == 0), stop=(kc == 3))
                    nc.vector.scalar_tensor_tensor(xT[:, dc, :], y_ps[:], bo_sb[:, dc:dc + 1],
                                                   xT[:, dc, :], ALU.add, ALU.add)

                # ---- LN2 + FFN ----
                h2T = sb2.tile([128, 4, 512], bf16, tag="hT", name=f"h2T_{l}")
                layer_norm(ln2g.ap()[l], ln2b.ap()[l], h2T)

                w1_sb = wp.tile([128, 4, FF], bf16, tag="w1", name=f"w1_{l}")
                nc.sync.dma_start(w1_sb[:], w1.ap()[l].rearrange("c p m -> p c m"))
                w2_sb = wp.tile([128, 16, 512], bf16, tag="w2", name=f"w2_{l}")
                nc.sync.dma_start(w2_sb[:], w2.ap()[l].rearrange("c p m -> p c m"))
                b1_sb = sb.tile([128, 16], f32, tag="b1", name=f"b1_{l}")
                nc.sync.dma_start(b1_sb[:], b1_in.ap()[l].rearrange("c p -> p c"))
                b2_sb = sb.tile([128, 4], f32, tag="bo", name=f"b2_{l}")
                nc.sync.dma_start(b2_sb[:], b2_in.ap()[l].rearrange("c p -> p c"))

                zT = sb2.tile([128, 16, 512], bf16, tag="zT", name=f"zT_{l}", bufs=1)
                for fc in range(16):
                    z_ps = ps.tile([128, 512], f32, tag="big", name=f"z_{l}_{fc}")
                    for kc in range(4):
                        nc.tensor.matmul(z_ps[:], w1_sb[:, kc, fc * 128:(fc + 1) * 128],
                                         h2T[:, kc, :], start=(kc == 0), stop=(kc == 3))
                    nc.scalar.activation(zT[:, fc, :], z_ps[:], func=AF.Relu,
                                         bias=b1_sb[:, fc:fc + 1])
                for dc in range(4):
                    y2_ps = ps.tile([128, 512], f32, tag="big", name=f"y2_{l}_{dc}")
                    for fc in range(16):
                        nc.tensor.matmul(y2_ps[:], w2_sb[:, fc, dc * 128:(dc + 1) * 128],
                                         zT[:, fc, :], start=(fc == 0), stop=(fc == 15))
                    nc.vector.scalar_tensor_tensor(xT[:, dc, :], y2_ps[:], b2_sb[:, dc:dc + 1],
                                                   xT[:, dc, :], ALU.add, ALU.add)

            # ---------------- final LN + AllGather ----------------
            hfT = sb2.tile([128, 4, 512], bf16, tag="hT", name="hfT")
            layer_norm(lnfg.ap(), lnfb.ap(), hfT)
            ag_in = dram.tile([512, 512], bf16, tag="ag_in", name="ag_in")
            nc.sync.dma_start(ag_in.rearrange("(c p) t -> p c t", p=128), hfT[:])
            ag_out = dram.tile([8, 512, 512], bf16, tag="ag_out", name="ag_out")
            nc.gpsimd.collective_compute("AllGather", ALU.bypass, replica_groups=GRP,
                                         ins=[ag_in.opt()], outs=[ag_out.opt()])
            trunk_ctx.close()

            # ---------------- LM head ----------------
            lmp = ctx.enter_context(tc.tile_pool(name="lmp", bufs=1))
            lm_sb = ctx.enter_context(tc.tile_pool(name="lm_sb", bufs=2))
            xf = lmp.tile([128, 4, 8, 512], bf16)
            for dc in range(4):
                nc.sync.dma_start(
                    xf[:, dc], ag_out[:, dc * 128:(dc + 1) * 128, :]
                    .rearrange("s p t -> p s t"))
            bout_bf = lmp.tile([1, VS], bf16)
            nc.sync.dma_start(bout_bf[:], bout_in.ap().rearrange("(o v) -> o v", o=1))
            se_sb = lmp.tile([128, 32], f32)
            nc.vector.memset(se_sb[:], 0.0)

            for vc in range(8):
                wo_vc = lm_sb.tile([128, 4, VC], bf16, tag="woutc", name=f"woutc_{vc}", bufs=2)
                nc.sync.dma_start(wo_vc[:], wout.ap()[:, :, vc * VC:(vc + 1) * VC]
                                  .rearrange("c p m -> p c m"))
                for m in range(32):
                    src, off = m // 4, (m % 4) * 128
                    lg_ps = ps.tile([128, 512], f32, tag="big", name=f"lg_{m}_{vc}")
                    for dc in range(4):
                        nc.tensor.matmul(lg_ps[:, :VC], xf[:, dc, src, off:off + 128],
                                         wo_vc[:, dc, :],
                                         start=(dc == 0), stop=False)
                    nc.tensor.matmul(lg_ps[:, :VC], ones_row_bf[:],
                                     bout_bf[:, vc * VC:(vc + 1) * VC],
                                     start=False, stop=True)
                    lo_sb = lm_sb.tile([128, VC], f32, tag="lo", name=f"lo_{m}_{vc}", bufs=3)
                    nc.vector.tensor_copy(lo_sb[:], lg_ps[:, :VC])
                    nc.sync.dma_start(
                        logits_out.ap()[m * 128:(m + 1) * 128, vc * VC:(vc + 1) * VC],
                        lo_sb[:])
                    ex_sb = lm_sb.tile([128, VC], bf16, tag="ex", name=f"ex_{m}_{vc}")
                    acc = lm_sb.tile([128, 1], f32, tag="acc", name=f"acc_{m}_{vc}", bufs=4)
                    nc.scalar.activation(ex_sb[:], lg_ps[:, :VC], func=AF.Exp,
                                         accum_out=acc[:])
                    nc.vector.tensor_add(se_sb[:, m:m + 1], se_sb[:, m:m + 1], acc[:])
            nc.sync.dma_start(sumexp_out.ap().rearrange("m p -> p m"), se_sb[:])

    nc.compile()
    return nc


# ---------------------------------------------------------------------------
# host-side input prep / output assembly
# ---------------------------------------------------------------------------

def _gather_wrap(token_ids):
    """Prepare the int16 index tensor for dma_gather so that the SBUF result
    lands token-major: out[p, i, :] = emb[token_ids[128*i + p]].

    Empirically the k-th gather descriptor is read from wrapped[k//32, k%32]
    and lands at out[16*(k%8) + k//32, (k//8)%4]."""
    k = np.arange(TOK)
    p = 16 * (k % 8) + k // 32
    i = (k // 8) % 4
    flat = token_ids[128 * i + p]
    return np.tile(flat.reshape(16, TOK // 16), (8, 1)).astype(np.int16)


def _prep_inputs(idx, targets, tok_emb, pos_emb, ln1_g, ln1_b, Wq, Wk, Wv, Wo, bo,
                 ln2_g, ln2_b, W1, b1, W2, b2, lnf_g, lnf_b, Wout, bout):
    bfl = ml_dtypes.bfloat16
    f32a = lambda a: np.ascontiguousarray(np.asarray(a), dtype=np.float32)
    bfa = lambda a: np.ascontiguousarray(np.asarray(a, dtype=np.float32).astype(bfl))

    shared = {
        "tok_emb": f32a(tok_emb),
        "wq": bfa(np.transpose(np.asarray(Wq), (0, 2, 1, 3)).reshape(L, 4, 128, H * HS)),
        "wk": bfa(np.transpose(np.asarray(Wk), (0, 2, 1, 3)).reshape(L, 4, 128, H * HS)),
        "wv": bfa(np.transpose(np.asarray(Wv), (0, 2, 1, 3)).reshape(L, 4, 128, H * HS)),
        "wo": bfa(np.asarray(Wo).reshape(L, 4, 128, D)),
        "w1": bfa(np.asarray(W1).reshape(L, 4, 128, FF)),
        "w2": bfa(np.asarray(W2).reshape(L, 16, 128, D)),
        "ln1g": f32a(ln1_g).reshape(L, 4, 128),
        "ln1b": f32a(ln1_b).reshape(L, 4, 128),
        "ln2g": f32a(ln2_g).reshape(L, 4, 128),
        "ln2b": f32a(ln2_b).reshape(L, 4, 128),
        "lnfg": f32a(lnf_g).reshape(4, 128),
        "lnfb": f32a(lnf_b).reshape(4, 128),
        "bo_in": f32a(bo).reshape(L, 4, 128),
        "b1_in": f32a(b1).reshape(L, 16, 128),
        "b2_in": f32a(b2).reshape(L, 4, 128),
    }
    mm = np.zeros((4, 128, 512), np.float32)
    for m in range(4):
        s = 128 * m + np.arange(128)[:, None]
        q = np.arange(512)[None, :]
        mm[m] = (s <= q).astype(np.float32)
    shared["masks_in"] = mm.astype(bfl)

    idx_flat = np.asarray(idx).reshape(-1).astype(np.int64)
    pos = np.asarray(pos_emb, dtype=np.float32)
    Wout_np = np.asarray(Wout, dtype=np.float32)
    bout_np = np.asarray(bout, dtype=np.float32)

    in_maps = []
    for c in range(NCORES):
        tok_ids = idx_flat[c * TOK:(c + 1) * TOK]
        pos_rows = pos[(c % 4) * TOK:(c % 4 + 1) * TOK, :]      # [512, D]
        pos_tm = pos_rows.reshape(4, 128, D).transpose(1, 0, 2)  # [128,4,D]
        m = dict(shared)
        m["idxw"] = _gather_wrap(tok_ids)
        m["pos_tm"] = np.ascontiguousarray(pos_tm)
        m["wout"] = np.ascontiguousarray(
            Wout_np[:, c * VS:(c + 1) * VS].reshape(4, 128, VS).astype(bfl))
        m["bout_in"] = np.ascontiguousarray(bout_np[c * VS:(c + 1) * VS].astype(bfl))
        in_maps.append(m)
    return in_maps


def get_program():
    if "nc" not in _CACHE:
        _CACHE["nc"] = _build()
    return _CACHE["nc"]


def run_device(in_maps, **kwargs):
    nc = get_program()
    return run_bass_kernel_spmd(nc, in_maps, core_ids=list(range(NCORES)), **kwargs)


def assemble(results, targets):
    logits = np.concatenate([r["logits_out"] for r in results], axis=1)
    sumexp = np.zeros(B * T, np.float64)
    for r in results:
        sumexp += r["sumexp_out"].reshape(32, 128).astype(np.float64).reshape(-1)
    lse = np.log(sumexp).astype(np.float32)
    tgt = np.asarray(targets).reshape(-1).astype(np.int64)
    tgt_logit = logits[np.arange(B * T), tgt]
    loss = np.float32(np.mean(lse - tgt_logit))
    return np.array(loss, dtype=np.float32), logits.reshape(B, T, V)


def kernel(**inputs):
    in_maps = _prep_inputs(**inputs)
    res = run_device(in_maps)
    return assemble(res.results, inputs["targets"])
